# revision 30
# baseline (speedup 1.0000x reference)
"""CliqueEncoder kernel for Trainium2 (8 NeuronCores, data-parallel).

Both columns of clique_attr are integers in [0, 4), so each output row
depends only on idx = 4*type + size -- 16 possible rows.  We fold
emb_table / W / b / gaussian basis into a 16 x 128 table on the host and
the device kernel is a pure 16-way row expansion over 1M rows.

Pipeline (vs the original row-major fp32 kernel, 220 us -> ~95-100 us):
  * Output is produced in bf16 (harness gate is rel_err < 2e-2; bf16
    rounding of the folded table is < 2e-3), halving HBM write traffic to
    the per-core minimum ~32 MB.
  * Feature-major packed layout: each fp32 PSUM word holds the bf16 PAIR
    [bf16(out[2hp]) | bf16(out[2hp+1])], built EXACTLY by three
    accumulating bf16 matmuls per chunk (widen(a) + scaled hi/lo bytes of
    b -- 24-bit significand, no rounding; verified on HW).  One [128,512]
    fp32 PSUM bank therefore holds 2048 rows x 128 features, halving PSUM
    evacuation (fp32-PSUM reads are 1x rate on DVE/ACT) while keeping all
    matmuls at full bf16 stream rate.
  * Per 2048-row tile: one K=124 "replication" matmul broadcasts the four
    512-row idx chunks onto the four 32-partition groups; one DVE
    is_equal builds the one-hot; 12 accumulating K=32 expansion matmuls
    (tile_position row+col tiling, j-waves so the four row strips stream
    concurrently); two [128,512] PSUM->SBUF copies (mostly ACT) stage the
    bits; 1 MiB HWDGE DMAs per 2 tiles write DRAM.
  * The next tile's replication+one-hot are emitted BEFORE this tile's
    expansion matmuls (software pipelining) so the strict-FIFO PE queue
    never stalls on the DVE.
  * idx (125 KB fp8) and ejs (0.5 MB fp8) minimize DMA-in; the final DMA
    group is trimmed to the columns covering real rows (pad rows skipped).

Per-core HBM traffic ~31.7 MB at the measured ~360 GB/s -> ~88 us floor;
TensorE ~53 us; ACT ~57 us; DVE ~47 us, all overlapped behind the DMA.
"""

import sys

sys.path.insert(0, "/opt/trn_rl_repo")

from contextlib import ExitStack

import numpy as np

# ---------------------------------------------------------------- constants
N = 1_000_000
H = 128
RBF = 32
H2 = H - H // 2  # 64
MAX_DIST = 20.0
NUM_TYPES = 4

N_CORES = 8
ROWS_PER_CORE = N // N_CORES  # 125000

F = 512  # rows per partition-chunk of a supertile
TILE_ROWS = 2048  # rows per expansion tile (4 chunks x 512)
GROUPS = 4  # partition groups of 32 per tile

P_SUPER = 124  # idx partitions per supertile (ejs sources 4t+g <= 123)
TILES_PER_SUPER = 31
N_SUPER = 2
ROWS_SUPER = P_SUPER * F  # 63488
ROWS_PAD = N_SUPER * ROWS_SUPER  # 126976
N_TILES = N_SUPER * TILES_PER_SUPER  # 62
OUT_COLS = N_TILES * 1024  # 63488 packed fp32 words per partition
# columns actually needed to cover ROWS_PER_CORE logical rows: full tiles
# 0..60 plus j < 72 of tile 61 chunk 0 (rest of tile 61 is padding)
OUT_COLS_USED = 61 * 1024 + (ROWS_PER_CORE - 61 * TILE_ROWS)  # 62536

# every Nth PSUM->SBUF copy goes to DVE, the rest to ACT.  ACT alone fits
# under the per-tile DMA cadence; DVE mostly just does the is_equal, so
# route only an occasional copy there to shorten PSUM-bank hold times.
DVE_COPY_EVERY = 4


def _bf16(x):
    import ml_dtypes

    return np.asarray(x).astype(ml_dtypes.bfloat16)


def _fp8(x):
    import ml_dtypes

    return np.asarray(x).astype(ml_dtypes.float8_e4m3)


# ------------------------------------------------------------- host tables
def _build_table16(emb_table, W, b):
    """table16[4*t + d] = concat(emb_table[t], basis(d) @ W[t] + b[t]).

    Computed with jax on CPU mirroring the reference ops exactly.
    """
    import jax
    import jax.numpy as jnp

    cpu = jax.local_devices(backend="cpu")[0]
    with jax.default_device(cpu):
        emb_table = jnp.asarray(np.asarray(emb_table, np.float32))
        W = jnp.asarray(np.asarray(W, np.float32))
        b = jnp.asarray(np.asarray(b, np.float32))
        centers = jnp.linspace(0.0, MAX_DIST, RBF)
        std = centers[1] - centers[0]
        d = jnp.arange(NUM_TYPES, dtype=jnp.float32)
        diff = d[:, None] - centers[None, :]
        basis = jnp.exp(-0.5 * diff * diff / (std * std))  # [4, RBF]
        rows = []
        for t in range(NUM_TYPES):
            size_emb = basis @ W[t] + b[t]  # [4, H2]
            for dd in range(NUM_TYPES):
                rows.append(jnp.concatenate([emb_table[t], size_emb[dd]]))
        table = np.asarray(jnp.stack(rows), np.float32)
    return table


def _build_consts(table16):
    """tblpk [128, 3*64] bf16 3-term packing tables, ejs fp8, iota.

    The device accumulates, per packed fp32 PSUM word targeting bf16 pair
    (a, b) = (bf16(out[2hp]), bf16(out[2hp+1])):
        psum = widen(a) + s_a*2^(e_a-142)*hibyte(b) + s_a*2^(e_a-150)*lobyte(b)
    Every term is exactly representable in bf16 and the fp32 sum is exactly
    the bit-concat [a | b] (24-bit significand; verified in numpy and the
    fp32-matmul probe).  This keeps all matmuls at full bf16 stream rate.
    """
    import ml_dtypes

    t16 = np.asarray(table16, np.float32)
    a = t16[:, 0::2].astype(ml_dtypes.bfloat16)
    b = t16[:, 1::2].astype(ml_dtypes.bfloat16)
    ab = a.view(np.uint16).astype(np.uint32)
    bb = b.view(np.uint16).astype(np.uint32)
    e_a = ((ab >> 7) & 0xFF).astype(np.int64)
    # normal-exponent guard: widen(a) must be a normal fp32 and the scaled
    # byte terms must stay in bf16 normal range
    assert e_a.min() > 24 and e_a.max() < 255, "packing hits denormal/inf"
    s_a = np.where((ab >> 15) & 1, -1.0, 1.0)
    t_hi = (s_a * np.ldexp((bb >> 8).astype(np.float64), e_a - 142)).astype(
        ml_dtypes.bfloat16
    )
    t_lo = (s_a * np.ldexp((bb & 0xFF).astype(np.float64), e_a - 150)).astype(
        ml_dtypes.bfloat16
    )
    # verify exact reconstruction under fp32 accumulation order a, hi, lo
    psum = a.astype(np.float32) + t_hi.astype(np.float32)
    psum = psum + t_lo.astype(np.float32)
    target = ((ab << 16) | bb).view(np.float32)
    assert np.array_equal(
        psum.view(np.uint32), target.view(np.uint32)
    ), "3-term packing not exact"

    tblpk = np.zeros((128, 3 * H2), ml_dtypes.bfloat16)
    for g in range(GROUPS):
        for j, term in enumerate((a, t_hi, t_lo)):
            tblpk[32 * g : 32 * g + 16, H2 * j : H2 * (j + 1)] = term

    ejs = np.zeros((P_SUPER, TILES_PER_SUPER * 128), np.float32)
    for t in range(TILES_PER_SUPER):
        for m in range(128):
            ejs[4 * t + m // 32, t * 128 + m] = 1.0
    ejs = _fp8(ejs)

    iota = (np.arange(128) % 32).astype(np.float32)[:, None]
    return tblpk, ejs, iota


def make_in_maps(clique_attr, emb_table, W, b):
    """Shard host-side inputs for the 8 cores."""
    attr = np.ascontiguousarray(np.asarray(clique_attr, np.int32))
    table16 = _build_table16(emb_table, W, b)
    tblpk, ejs, iota = _build_consts(table16)
    idx_all = (4 * attr[:, 0] + attr[:, 1]).astype(np.float32)
    in_maps = []
    for c in range(N_CORES):
        sl = idx_all[c * ROWS_PER_CORE : (c + 1) * ROWS_PER_CORE]
        pad = np.zeros(ROWS_PAD, np.float32)
        pad[: len(sl)] = sl
        in_maps.append(
            {"idx": _fp8(pad), "tblpk": tblpk, "ejs": ejs, "iota": iota}
        )
    return in_maps


# ------------------------------------------------------------ bass builder
def build_nc(
    reps=None,
    internal_io=False,
    # full | dma_only | no_out_dma | no_copies | exp_only | no_exp
    mode="full",
    dma_tiles=2,  # expansion tiles per output DMA (x 512 KiB each)
):
    """Build the bass kernel.

    reps/internal_io are for hardware timing only: idx/out become Internal
    DRAM tensors and the whole body is wrapped in a hardware For_i loop.
    """
    import concourse.bacc as bacc
    import concourse.bass as bass
    import concourse.mybir as mybir
    import concourse.tile as tile

    f32 = mybir.dt.float32
    bf16 = mybir.dt.bfloat16
    fp8 = mybir.dt.float8e4

    nc = bacc.Bacc(None, target_bir_lowering=False)

    io_kind = "Internal" if internal_io else None
    idx_d = nc.dram_tensor(
        "idx", [ROWS_PAD], fp8, kind=io_kind or "ExternalInput"
    )
    tbl_d = nc.dram_tensor("tblpk", [128, 3 * H2], bf16, kind="ExternalInput")
    ejs_d = nc.dram_tensor(
        "ejs", [P_SUPER, TILES_PER_SUPER * 128], fp8, kind="ExternalInput"
    )
    iota_d = nc.dram_tensor("iota", [128, 1], f32, kind="ExternalInput")
    # Packed feature-major output: out[64*half + hp, 1024*t + 512*ab + j]
    # holds bf16 pair (h = 2hp, 2hp+1) of logical row
    # 2048*t + 1024*ab + 512*half + j.
    out_d = nc.dram_tensor(
        "out", [128, OUT_COLS], f32, kind=io_kind or "ExternalOutput"
    )
    dummy_d = (
        nc.dram_tensor("probe", [128, 3 * H2], bf16, kind="ExternalOutput")
        if internal_io
        else None
    )

    with tile.TileContext(nc) as tc, ExitStack() as ctx:
        const_p = ctx.enter_context(tc.tile_pool(name="const", bufs=1))
        idx_p = ctx.enter_context(tc.tile_pool(name="idx", bufs=2))
        oh_p = ctx.enter_context(tc.tile_pool(name="oh", bufs=4))
        out_p = ctx.enter_context(tc.tile_pool(name="out", bufs=4))
        psi_p = ctx.enter_context(
            tc.tile_pool(name="psi", bufs=3, space=bass.MemorySpace.PSUM)
        )
        pso_p = ctx.enter_context(
            tc.tile_pool(name="pso", bufs=4, space=bass.MemorySpace.PSUM)
        )

        tbl = const_p.tile([128, 3 * H2], bf16)
        nc.sync.dma_start(tbl[:], tbl_d[:, :])
        ejs = const_p.tile([P_SUPER, TILES_PER_SUPER * 128], fp8)
        nc.sync.dma_start(ejs[:], ejs_d[:, :])
        iota = const_p.tile([128, 1], f32)
        nc.sync.dma_start(iota[:], iota_d[:, :])
        oh_const = None
        if mode == "exp_only":
            oh_const = const_p.tile([128, F], bf16)
            nc.vector.memset(oh_const[:], 0.0)

        def emit_body():
            idx_tiles = []
            for s in range(N_SUPER):
                idx_sb = idx_p.tile([P_SUPER, F], fp8, name=f"idx_{s}")
                nc.sync.dma_start(
                    idx_sb[:],
                    idx_d[s * ROWS_SUPER : (s + 1) * ROWS_SUPER].rearrange(
                        "(p f) -> p f", p=P_SUPER
                    ),
                )
                idx_tiles.append(idx_sb)

            def make_oh(gt):
                """Replication matmul + one-hot for tile gt."""
                s, t = divmod(gt, TILES_PER_SUPER)
                ps_idx = psi_p.tile([128, F], f32)
                nc.tensor.matmul(
                    ps_idx[:],
                    ejs[:, t * 128 : (t + 1) * 128],
                    idx_tiles[s][:],
                    start=True,
                    stop=True,
                )
                oh = oh_p.tile([128, F], bf16)
                nc.vector.tensor_scalar(
                    oh[:], ps_idx[:], iota[:], None, mybir.AluOpType.is_equal
                )
                return oh

            out_sb = None
            oh_next = None
            if mode in ("full", "no_out_dma", "no_copies", "no_exp"):
                oh_next = make_oh(0)
            for gt in range(N_TILES):
                slot = gt % dma_tiles
                if slot == 0 and mode in ("full", "dma_only", "no_out_dma"):
                    out_sb = out_p.tile([128, 1024 * dma_tiles], f32)
                    if mode == "dma_only":
                        nc.vector.memset(out_sb[:, 0:4], 0.0)

                if mode != "dma_only":
                    # software pipeline: next tile's replication + one-hot are
                    # emitted BEFORE this tile's expansion matmuls, so the PE
                    # FIFO never stalls waiting on the DVE is_equal
                    if mode == "exp_only":
                        oh = oh_const
                    else:
                        oh = oh_next
                        if gt + 1 < N_TILES:
                            oh_next = make_oh(gt + 1)
                    if mode == "no_exp":
                        continue
                    # last tile: only chunks 0/1 (psA) cover needed rows
                    last = gt == N_TILES - 1
                    n_ab = 1 if last else 2
                    ps_ab = [
                        pso_p.tile([128, F], f32, tag="pso", name=f"ps{ab}")
                        for ab in range(n_ab)
                    ]
                    # 3 accumulating bf16 matmuls per chunk build the packed
                    # [bf16|bf16] fp32 word exactly; j-waves across the four
                    # row strips so strips stream concurrently
                    for j in range(3):
                        for g in range(2 * n_ab):
                            half = g % 2
                            nc.tensor.matmul(
                                ps_ab[g // 2][64 * half : 64 * half + 64, :],
                                tbl[32 * g : 32 * g + 32, H2 * j : H2 * (j + 1)],
                                oh[32 * g : 32 * g + 32, :],
                                start=(j == 0),
                                stop=(j == 2),
                                tile_position=(32 * g, 64 * half),
                            )
                    if mode not in ("no_copies", "exp_only"):
                        for ab in range(n_ab):
                            dst = out_sb[
                                :,
                                1024 * slot + 512 * ab : 1024 * slot + 512 * ab + 512,
                            ]
                            if (2 * gt + ab) % DVE_COPY_EVERY == 0:
                                nc.vector.tensor_copy(dst, ps_ab[ab][:])
                            else:
                                nc.scalar.copy(dst, ps_ab[ab][:])

                if mode in ("full", "dma_only") and slot == dma_tiles - 1:
                    c0 = (gt - slot) * 1024
                    w = min(1024 * dma_tiles, OUT_COLS_USED - c0)
                    nc.sync.dma_start(
                        out_d[:, c0 : c0 + w], out_sb[:, :w]
                    )

        if reps is None:
            emit_body()
        else:
            with tc.For_i(0, reps, 1, hint_engines=tuple(mybir.ALL_ENGINES)):
                emit_body()

        if dummy_d is not None:
            nc.sync.dma_start(dummy_d[:, :], tbl[:])

    nc.compile()
    return nc


# --------------------------------------------------------------- host entry
_CACHE = {}


def _get_nc():
    if "nc" not in _CACHE:
        _CACHE["nc"] = build_nc()
    return _CACHE["nc"]


def _unshard(dev):
    """[128, OUT_COLS] packed fp32 -> [ROWS_PER_CORE, H] fp32."""
    import ml_dtypes

    v = np.ascontiguousarray(dev).view(np.uint32)
    v = v.reshape(2, 64, N_TILES, 2, F)  # [half, hp, t, ab, j]
    hi = (v >> np.uint32(16)).astype(np.uint16)
    lo = (v & np.uint32(0xFFFF)).astype(np.uint16)
    hl = np.stack([hi, lo], axis=-1)  # [half, hp, t, ab, j, 2]
    rows = hl.transpose(2, 3, 0, 4, 1, 5).reshape(ROWS_PAD, H)
    return (
        rows[:ROWS_PER_CORE].view(ml_dtypes.bfloat16).astype(np.float32)
    )


def kernel(clique_attr, emb_table, W, b):
    from concourse.bass_utils import run_bass_kernel_spmd

    in_maps = make_in_maps(clique_attr, emb_table, W, b)
    nc = _get_nc()
    res = run_bass_kernel_spmd(nc, in_maps, core_ids=list(range(N_CORES)))
    out = np.empty((N, H), np.float32)
    for c in range(N_CORES):
        dev = np.asarray(res.results[c]["out"], np.float32)
        out[c * ROWS_PER_CORE : (c + 1) * ROWS_PER_CORE] = _unshard(dev)
    return out


# revision 35
# speedup vs baseline: 1.4998x; 1.4998x over previous
"""CliqueEncoder kernel for Trainium2 (8 NeuronCores, data-parallel).

Both columns of clique_attr are integers in [0, 4), so each output row
depends only on idx = 4*type + size -- 16 possible rows.  We fold
emb_table / W / b / gaussian basis into a 16 x 128 table on the host and
the device kernel is a pure 16-way row expansion over 1M rows.

Pipeline (vs the original row-major fp32 kernel, 220 us -> ~95-100 us):
  * Output is produced in bf16 (harness gate is rel_err < 2e-2; bf16
    rounding of the folded table is < 2e-3), halving HBM write traffic to
    the per-core minimum ~32 MB.
  * Feature-major packed layout: each fp32 PSUM word holds the bf16 PAIR
    [bf16(out[2hp]) | bf16(out[2hp+1])], built EXACTLY by three
    accumulating bf16 matmuls per chunk (widen(a) + scaled hi/lo bytes of
    b -- 24-bit significand, no rounding; verified on HW).  One [128,512]
    fp32 PSUM bank therefore holds 2048 rows x 128 features, halving PSUM
    evacuation (fp32-PSUM reads are 1x rate on DVE/ACT) while keeping all
    matmuls at full bf16 stream rate.
  * Per 2048-row tile: one K=124 "replication" matmul broadcasts the four
    512-row idx chunks onto the four 32-partition groups; one DVE
    is_equal builds the one-hot; 12 accumulating K=32 expansion matmuls
    (tile_position row+col tiling, j-waves so the four row strips stream
    concurrently); two [128,512] PSUM->SBUF copies (mostly ACT) stage the
    bits; 1 MiB HWDGE DMAs per 2 tiles write DRAM.
  * The next tile's replication+one-hot are emitted BEFORE this tile's
    expansion matmuls (software pipelining) so the strict-FIFO PE queue
    never stalls on the DVE.
  * idx (125 KB fp8) and ejs (0.5 MB fp8) minimize DMA-in; the final DMA
    group is trimmed to the columns covering real rows (pad rows skipped).

Per-core HBM traffic ~31.7 MB at the measured ~360 GB/s -> ~88 us floor;
TensorE ~53 us; ACT ~57 us; DVE ~47 us, all overlapped behind the DMA.
"""

import sys

sys.path.insert(0, "/opt/trn_rl_repo")

from contextlib import ExitStack

import numpy as np

# ---------------------------------------------------------------- constants
N = 1_000_000
H = 128
RBF = 32
H2 = H - H // 2  # 64
MAX_DIST = 20.0
NUM_TYPES = 4

N_CORES = 8
ROWS_PER_CORE = N // N_CORES  # 125000

F = 512  # rows per partition-chunk of a supertile
TILE_ROWS = 2048  # rows per expansion tile (4 chunks x 512)
GROUPS = 4  # partition groups of 32 per tile

P_SUPER = 124  # idx partitions per supertile (ejs sources 4t+g <= 123)
TILES_PER_SUPER = 31
N_SUPER = 2
ROWS_SUPER = P_SUPER * F  # 63488
ROWS_PAD = N_SUPER * ROWS_SUPER  # 126976
N_TILES = N_SUPER * TILES_PER_SUPER  # 62
OUT_COLS = N_TILES * 1024  # 63488 packed fp32 words per partition
# columns actually needed to cover ROWS_PER_CORE logical rows: full tiles
# 0..60 plus j < 72 of tile 61 chunk 0 (rest of tile 61 is padding)
OUT_COLS_USED = 61 * 1024 + (ROWS_PER_CORE - 61 * TILE_ROWS)  # 62536

# every Nth PSUM->SBUF copy goes to DVE, the rest to ACT.  At N=5 the two
# engines' per-tile streams balance: DVE is_equal 658 + 0.4*658 = 921 ns,
# ACT 1.6 * 570 = 912 ns (minimizes the max-engine evacuation bound).
DVE_COPY_EVERY = 5


def _bf16(x):
    import ml_dtypes

    return np.asarray(x).astype(ml_dtypes.bfloat16)


def _fp8(x):
    import ml_dtypes

    return np.asarray(x).astype(ml_dtypes.float8_e4m3)


# ------------------------------------------------------------- host tables
def _build_table16(emb_table, W, b):
    """table16[4*t + d] = concat(emb_table[t], basis(d) @ W[t] + b[t]).

    Computed with jax on CPU mirroring the reference ops exactly.
    """
    import jax
    import jax.numpy as jnp

    cpu = jax.local_devices(backend="cpu")[0]
    with jax.default_device(cpu):
        emb_table = jnp.asarray(np.asarray(emb_table, np.float32))
        W = jnp.asarray(np.asarray(W, np.float32))
        b = jnp.asarray(np.asarray(b, np.float32))
        centers = jnp.linspace(0.0, MAX_DIST, RBF)
        std = centers[1] - centers[0]
        d = jnp.arange(NUM_TYPES, dtype=jnp.float32)
        diff = d[:, None] - centers[None, :]
        basis = jnp.exp(-0.5 * diff * diff / (std * std))  # [4, RBF]
        rows = []
        for t in range(NUM_TYPES):
            size_emb = basis @ W[t] + b[t]  # [4, H2]
            for dd in range(NUM_TYPES):
                rows.append(jnp.concatenate([emb_table[t], size_emb[dd]]))
        table = np.asarray(jnp.stack(rows), np.float32)
    return table


def _build_consts(table16):
    """tblpk [128, 3*64] bf16 3-term packing tables, ejs fp8, iota.

    The device accumulates, per packed fp32 PSUM word targeting bf16 pair
    (a, b) = (bf16(out[2hp]), bf16(out[2hp+1])):
        psum = widen(a) + s_a*2^(e_a-142)*hibyte(b) + s_a*2^(e_a-150)*lobyte(b)
    Every term is exactly representable in bf16 and the fp32 sum is exactly
    the bit-concat [a | b] (24-bit significand; verified in numpy and the
    fp32-matmul probe).  This keeps all matmuls at full bf16 stream rate.
    """
    import ml_dtypes

    t16 = np.asarray(table16, np.float32)
    a = t16[:, 0::2].astype(ml_dtypes.bfloat16)
    b = t16[:, 1::2].astype(ml_dtypes.bfloat16)
    ab = a.view(np.uint16).astype(np.uint32)
    bb = b.view(np.uint16).astype(np.uint32)
    e_a = ((ab >> 7) & 0xFF).astype(np.int64)
    # normal-exponent guard: widen(a) must be a normal fp32 and the scaled
    # byte terms must stay in bf16 normal range
    assert e_a.min() > 24 and e_a.max() < 255, "packing hits denormal/inf"
    s_a = np.where((ab >> 15) & 1, -1.0, 1.0)
    t_hi = (s_a * np.ldexp((bb >> 8).astype(np.float64), e_a - 142)).astype(
        ml_dtypes.bfloat16
    )
    t_lo = (s_a * np.ldexp((bb & 0xFF).astype(np.float64), e_a - 150)).astype(
        ml_dtypes.bfloat16
    )
    # verify exact reconstruction under fp32 accumulation order a, hi, lo
    psum = a.astype(np.float32) + t_hi.astype(np.float32)
    psum = psum + t_lo.astype(np.float32)
    target = ((ab << 16) | bb).view(np.float32)
    assert np.array_equal(
        psum.view(np.uint32), target.view(np.uint32)
    ), "3-term packing not exact"

    tblpk = np.zeros((128, 3 * H2), ml_dtypes.bfloat16)
    for g in range(GROUPS):
        for j, term in enumerate((a, t_hi, t_lo)):
            tblpk[32 * g : 32 * g + 16, H2 * j : H2 * (j + 1)] = term

    ejs = np.zeros((P_SUPER, TILES_PER_SUPER * 128), np.float32)
    for t in range(TILES_PER_SUPER):
        for m in range(128):
            ejs[4 * t + m // 32, t * 128 + m] = 1.0
    ejs = _fp8(ejs)

    iota = (np.arange(128) % 32).astype(np.float32)[:, None]
    return tblpk, ejs, iota


def make_in_maps(clique_attr, emb_table, W, b):
    """Shard host-side inputs for the 8 cores."""
    attr = np.ascontiguousarray(np.asarray(clique_attr, np.int32))
    table16 = _build_table16(emb_table, W, b)
    tblpk, ejs, iota = _build_consts(table16)
    idx_all = (4 * attr[:, 0] + attr[:, 1]).astype(np.float32)
    in_maps = []
    for c in range(N_CORES):
        sl = idx_all[c * ROWS_PER_CORE : (c + 1) * ROWS_PER_CORE]
        pad = np.zeros(ROWS_PAD, np.float32)
        pad[: len(sl)] = sl
        in_maps.append(
            {"idx": _fp8(pad), "tblpk": tblpk, "ejs": ejs, "iota": iota}
        )
    return in_maps


# ------------------------------------------------------------ bass builder
def build_nc(
    reps=None,
    internal_io=False,
    # full | dma_only | no_out_dma | no_copies | exp_only | no_exp
    mode="full",
    dma_tiles=2,  # expansion tiles per output DMA (x 512 KiB each)
    pso_bufs=5,  # PSUM banks for expansion outputs (2 per tile in flight)
    dual_ring=False,  # alternate out-DMA groups between sync and scalar HWDGE
):
    """Build the bass kernel.

    reps/internal_io are for hardware timing only: idx/out become Internal
    DRAM tensors and the whole body is wrapped in a hardware For_i loop.
    """
    import concourse.bacc as bacc
    import concourse.bass as bass
    import concourse.mybir as mybir
    import concourse.tile as tile

    f32 = mybir.dt.float32
    bf16 = mybir.dt.bfloat16
    fp8 = mybir.dt.float8e4

    nc = bacc.Bacc(None, target_bir_lowering=False)

    io_kind = "Internal" if internal_io else None
    idx_d = nc.dram_tensor(
        "idx", [ROWS_PAD], fp8, kind=io_kind or "ExternalInput"
    )
    tbl_d = nc.dram_tensor("tblpk", [128, 3 * H2], bf16, kind="ExternalInput")
    ejs_d = nc.dram_tensor(
        "ejs", [P_SUPER, TILES_PER_SUPER * 128], fp8, kind="ExternalInput"
    )
    iota_d = nc.dram_tensor("iota", [128, 1], f32, kind="ExternalInput")
    # Packed feature-major output: out[64*half + hp, 1024*t + 512*ab + j]
    # holds bf16 pair (h = 2hp, 2hp+1) of logical row
    # 2048*t + 1024*ab + 512*half + j.
    out_d = nc.dram_tensor(
        "out", [128, OUT_COLS], f32, kind=io_kind or "ExternalOutput"
    )
    dummy_d = (
        nc.dram_tensor("probe", [128, 3 * H2], bf16, kind="ExternalOutput")
        if internal_io
        else None
    )

    with tile.TileContext(nc) as tc, ExitStack() as ctx:
        const_p = ctx.enter_context(tc.tile_pool(name="const", bufs=1))
        idx_p = ctx.enter_context(tc.tile_pool(name="idx", bufs=2))
        oh_p = ctx.enter_context(tc.tile_pool(name="oh", bufs=4))
        out_p = ctx.enter_context(tc.tile_pool(name="out", bufs=4))
        psi_p = ctx.enter_context(
            tc.tile_pool(name="psi", bufs=3, space=bass.MemorySpace.PSUM)
        )
        pso_p = ctx.enter_context(
            tc.tile_pool(name="pso", bufs=pso_bufs, space=bass.MemorySpace.PSUM)
        )

        tbl = const_p.tile([128, 3 * H2], bf16)
        nc.sync.dma_start(tbl[:], tbl_d[:, :])
        ejs = const_p.tile([P_SUPER, TILES_PER_SUPER * 128], fp8)
        nc.sync.dma_start(ejs[:], ejs_d[:, :])
        iota = const_p.tile([128, 1], f32)
        nc.sync.dma_start(iota[:], iota_d[:, :])
        oh_const = None
        if mode == "exp_only":
            oh_const = const_p.tile([128, F], bf16)
            nc.vector.memset(oh_const[:], 0.0)

        def emit_body():
            idx_tiles = []
            for s in range(N_SUPER):
                idx_sb = idx_p.tile([P_SUPER, F], fp8, name=f"idx_{s}")
                nc.sync.dma_start(
                    idx_sb[:],
                    idx_d[s * ROWS_SUPER : (s + 1) * ROWS_SUPER].rearrange(
                        "(p f) -> p f", p=P_SUPER
                    ),
                )
                idx_tiles.append(idx_sb)

            def make_oh(gt):
                """Replication matmul + one-hot for tile gt."""
                s, t = divmod(gt, TILES_PER_SUPER)
                ps_idx = psi_p.tile([128, F], f32)
                nc.tensor.matmul(
                    ps_idx[:],
                    ejs[:, t * 128 : (t + 1) * 128],
                    idx_tiles[s][:],
                    start=True,
                    stop=True,
                )
                oh = oh_p.tile([128, F], bf16)
                nc.vector.tensor_scalar(
                    oh[:], ps_idx[:], iota[:], None, mybir.AluOpType.is_equal
                )
                return oh

            out_sb = None
            oh_next = None
            if mode in ("full", "no_out_dma", "no_copies", "no_exp"):
                oh_next = make_oh(0)
            for gt in range(N_TILES):
                slot = gt % dma_tiles
                if slot == 0 and mode in ("full", "dma_only", "no_out_dma"):
                    out_sb = out_p.tile([128, 1024 * dma_tiles], f32)
                    if mode == "dma_only":
                        nc.vector.memset(out_sb[:, 0:4], 0.0)

                if mode != "dma_only":
                    # software pipeline: next tile's replication + one-hot are
                    # emitted BEFORE this tile's expansion matmuls, so the PE
                    # FIFO never stalls waiting on the DVE is_equal
                    if mode == "exp_only":
                        oh = oh_const
                    else:
                        oh = oh_next
                        if gt + 1 < N_TILES:
                            oh_next = make_oh(gt + 1)
                    if mode == "no_exp":
                        continue
                    # last tile: only chunks 0/1 (psA) cover needed rows
                    last = gt == N_TILES - 1
                    n_ab = 1 if last else 2
                    ps_ab = [
                        pso_p.tile([128, F], f32, tag="pso", name=f"ps{ab}")
                        for ab in range(n_ab)
                    ]
                    # 3 accumulating bf16 matmuls per chunk build the packed
                    # [bf16|bf16] fp32 word exactly; j-waves across the four
                    # row strips so strips stream concurrently
                    for j in range(3):
                        for g in range(2 * n_ab):
                            half = g % 2
                            nc.tensor.matmul(
                                ps_ab[g // 2][64 * half : 64 * half + 64, :],
                                tbl[32 * g : 32 * g + 32, H2 * j : H2 * (j + 1)],
                                oh[32 * g : 32 * g + 32, :],
                                start=(j == 0),
                                stop=(j == 2),
                                tile_position=(32 * g, 64 * half),
                            )
                    if mode not in ("no_copies", "exp_only"):
                        for ab in range(n_ab):
                            dst = out_sb[
                                :,
                                1024 * slot + 512 * ab : 1024 * slot + 512 * ab + 512,
                            ]
                            if (2 * gt + ab) % DVE_COPY_EVERY == 0:
                                nc.vector.tensor_copy(dst, ps_ab[ab][:])
                            else:
                                nc.scalar.copy(dst, ps_ab[ab][:])

                if mode in ("full", "dma_only") and slot == dma_tiles - 1:
                    c0 = (gt - slot) * 1024
                    w = min(1024 * dma_tiles, OUT_COLS_USED - c0)
                    group = gt // dma_tiles
                    eng = nc.scalar if (dual_ring and group % 2) else nc.sync
                    eng.dma_start(out_d[:, c0 : c0 + w], out_sb[:, :w])

        if reps is None:
            emit_body()
        else:
            with tc.For_i(0, reps, 1, hint_engines=tuple(mybir.ALL_ENGINES)):
                emit_body()

        if dummy_d is not None:
            nc.sync.dma_start(dummy_d[:, :], tbl[:])

    nc.compile()
    return nc


# --------------------------------------------------------------- host entry
_CACHE = {}


def _get_nc():
    if "nc" not in _CACHE:
        _CACHE["nc"] = build_nc()
    return _CACHE["nc"]


def _unshard(dev):
    """[128, OUT_COLS] packed fp32 -> [ROWS_PER_CORE, H] fp32."""
    import ml_dtypes

    v = np.ascontiguousarray(dev).view(np.uint32)
    v = v.reshape(2, 64, N_TILES, 2, F)  # [half, hp, t, ab, j]
    hi = (v >> np.uint32(16)).astype(np.uint16)
    lo = (v & np.uint32(0xFFFF)).astype(np.uint16)
    hl = np.stack([hi, lo], axis=-1)  # [half, hp, t, ab, j, 2]
    rows = hl.transpose(2, 3, 0, 4, 1, 5).reshape(ROWS_PAD, H)
    return (
        rows[:ROWS_PER_CORE].view(ml_dtypes.bfloat16).astype(np.float32)
    )


def kernel(clique_attr, emb_table, W, b):
    from concourse.bass_utils import run_bass_kernel_spmd

    in_maps = make_in_maps(clique_attr, emb_table, W, b)
    nc = _get_nc()
    res = run_bass_kernel_spmd(nc, in_maps, core_ids=list(range(N_CORES)))
    out = np.empty((N, H), np.float32)
    for c in range(N_CORES):
        dev = np.asarray(res.results[c]["out"], np.float32)
        out[c * ROWS_PER_CORE : (c + 1) * ROWS_PER_CORE] = _unshard(dev)
    return out
